# revision 10
# baseline (speedup 1.0000x reference)
"""Trainium2 Bass kernel for KV-cached (causal) multi-head attention.

Full module: y = softmax(mask(QK^T/sqrt(hd))) V  -> out_proj, with
Q/K/V = linear projections of query/key/value inputs.

Shapes (hardcoded): B=2, S=2048, D=2048, H=16 heads, hd=128.

Sharding (8 NeuronCores): core c handles batch b=c//4 and head group
g=c%4 (4 heads = 512 dims).  Each core computes its 4 heads' attention
and a partial output projection y_partial = attn @ Wo[:, g].T; the host
sums the 4 partials per batch and adds bo (the "all-reduce after
out_proj" done host-side at gather time).

On-device layout choices (all matmuls bf16, fp32 PSUM accumulation):
  - host passes transposed activations xT [D, S] and weight slices
    WqT/WkT/WvT = W[g,:].T [D, 512], WoT = Wo[:, g].T [512, D]
  - Q^T, K^T computed as [dq, S] (head dim on partitions) so that
    scores = Q^T.T @ K^T needs no on-device transposes
  - V computed as [S, dv]
  - softmax per q-row (partition) along free kv axis; exp on ScalarE
    with fused per-chunk row-sums (accum_out); causal handled by
    skipping kv blocks beyond the diagonal + one additive mask tile
    on the diagonal 128x128 block
  - P^T for the PV matmul via PE-mode transposes of 128x128 blocks
  - attention output [q, hd] re-transposed per 128-block to feed the
    output projection as lhsT
"""

import sys

for _p in ("/opt/trn_rl_repo",):
    if _p not in sys.path:
        sys.path.insert(0, _p)

from contextlib import ExitStack

import numpy as np
import ml_dtypes

import concourse.bass as bass
import concourse.mybir as mybir
import concourse.tile as tile
from concourse.vector_clock import ScopedClock
from concourse.masks import make_causal_mask, make_identity

BF16 = mybir.dt.bfloat16
F32 = mybir.dt.float32
NP_BF16 = ml_dtypes.bfloat16

B, S, D = 2, 2048, 2048
NH, HD = 16, 128          # total heads, head dim
GH = 4                    # heads per core
GD = GH * HD              # 512 dims per core
P = 128
SCALE = 1.0 / np.sqrt(HD)
N_CORES = 8


def _drain_and_barrier_split(self, tick_clock, wait_clock):
    # The walrus build in this container rejects a Drain carrying more
    # than one sync wait ("Too many sync wait commands").  Semantically
    # equivalent: chain one drain per wait on the sync engine.
    nc = self.nc
    drain_inst = nc.sync.drain()
    wait_clock.add_sem_waits(
        drain_inst.ins, ScopedClock({None: tick_clock.global_clock})
    )
    si = drain_inst.ins.sync_info
    waits = list(si.on_wait)
    if len(waits) > 1:
        drain_inst.ins.sync_info = mybir.SyncInfo(
            on_wait=[waits[0]], on_update=list(si.on_update)
        )
        for w in waits[1:]:
            d = nc.sync.drain()
            d.ins.sync_info = mybir.SyncInfo(on_wait=[w], on_update=[])
    nc.all_engine_barrier()
    assert self.sems is not None
    popped = nc._tile_sem_poison_stack.pop()
    assert popped is self._sem_poison
    nc.clear_and_free_semaphores(list(self.sems.allocated().values()))
    nc.all_engine_barrier()


tile.TileContext._drain_and_barrier = _drain_and_barrier_split


def _split_multi_waits(nc, max_waits=1):
    """This container's walrus rejects instructions carrying more than one
    sync wait.  Hoist extra waits onto same-engine NoOps placed just before
    the instruction (waits execute in engine program order, so this is
    semantically identical)."""
    uid = [0]
    for fn in nc.m.functions:
        for bb in fn.blocks:
            insts = bb.instructions
            new = []
            changed = False
            for inst in insts:
                si = getattr(inst, "sync_info", None)
                waits = list(si.on_wait) if si is not None else []
                if len(waits) > max_waits:
                    changed = True
                    n_keep = max_waits
                    for w in waits[:-n_keep]:
                        nop = mybir.InstNoOp(
                            name=f"WSPLIT-{uid[0]}", ins=[], outs=[]
                        )
                        uid[0] += 1
                        nop.engine = inst.engine
                        nop.sync_info = mybir.SyncInfo(
                            on_wait=[w], on_update=[]
                        )
                        new.append(nop)
                    inst.sync_info = mybir.SyncInfo(
                        on_wait=waits[-n_keep:], on_update=list(si.on_update)
                    )
                new.append(inst)
            if changed:
                bb.instructions = new
    return nc


def build_bass():
    nc = bass.Bass()
    xqT = nc.declare_dram_parameter("xqT", [D, S], BF16, isOutput=False)
    xkT = nc.declare_dram_parameter("xkT", [D, S], BF16, isOutput=False)
    xvT = nc.declare_dram_parameter("xvT", [D, S], BF16, isOutput=False)
    wqT = nc.declare_dram_parameter("wqT", [D, GD], BF16, isOutput=False)
    wkT = nc.declare_dram_parameter("wkT", [D, GD], BF16, isOutput=False)
    wvT = nc.declare_dram_parameter("wvT", [D, GD], BF16, isOutput=False)
    woT = nc.declare_dram_parameter("woT", [GD, D], BF16, isOutput=False)
    bq4 = nc.declare_dram_parameter("bq4", [P, GH], F32, isOutput=False)
    bk4 = nc.declare_dram_parameter("bk4", [P, GH], F32, isOutput=False)
    bvb = nc.declare_dram_parameter("bvb", [P, GD], F32, isOutput=False)
    y = nc.declare_dram_parameter("y", [S, D], F32, isOutput=True)

    KC = D // P               # 16 contraction chunks of 128
    TT = S // 512             # 4 t-tiles of 512
    QI = S // P               # 16 q tiles of 128

    with tile.TileContext(nc) as tc, ExitStack() as ctx:
        const = ctx.enter_context(tc.tile_pool(name="const", bufs=1))
        maskt = const.tile([P, P], F32)
        make_causal_mask(nc, maskt, mask_val=-1e9)
        ident = const.tile([P, P], BF16)
        make_identity(nc, ident)
        bq_sb = const.tile([P, GH], F32)
        nc.sync.dma_start(bq_sb[:], bq4[:])
        bk_sb = const.tile([P, GH], F32)
        nc.sync.dma_start(bk_sb[:], bk4[:])
        bv_sb = const.tile([P, GD], F32)
        nc.sync.dma_start(bv_sb[:], bvb[:])

        # resident weights: 16 chunks of [128, 512] each
        wpool = ctx.enter_context(tc.tile_pool(name="weights", bufs=1))
        wq_sb, wk_sb, wv_sb = [], [], []
        for name, dram, lst in (
            ("wq", wqT, wq_sb), ("wk", wkT, wk_sb), ("wv", wvT, wv_sb)
        ):
            for kc in range(KC):
                t = wpool.tile([P, GD], BF16, name=f"{name}{kc}", tag=f"{name}{kc}")
                nc.sync.dma_start(t[:], dram[kc * P:(kc + 1) * P, :])
                lst.append(t)
        wo_sb = []
        for hb in range(GH):
            t = wpool.tile([P, D], BF16, name=f"woc{hb}", tag=f"wo{hb}")
            nc.sync.dma_start(t[:], woT[hb * P:(hb + 1) * P, :])
            wo_sb.append(t)

        # persistent activations
        act = ctx.enter_context(tc.tile_pool(name="acts", bufs=1))
        qT_sb = [act.tile([P, S], BF16, name=f"qT{h}", tag=f"qT{h}") for h in range(GH)]
        kT_sb = [act.tile([P, S], BF16, name=f"kT{h}", tag=f"kT{h}") for h in range(GH)]
        v_sb = [act.tile([P, GD], BF16, name=f"v{i}", tag=f"v{i}") for i in range(QI)]

        ctxA = ExitStack()
        xin = ctxA.enter_context(tc.tile_pool(name="xin", bufs=24))
        ps512 = ctx.enter_context(
            tc.tile_pool(name="ps512", bufs=4, space="PSUM")
        )

        # ---- Q^T / K^T projections: out [dq=512, S] ----
        for xdram, w_sb, out_tiles, b_tile, scale in (
            (xqT, wq_sb, qT_sb, bq_sb, SCALE),
            (xkT, wk_sb, kT_sb, bk_sb, 1.0),
        ):
            for tt in range(TT):
                xch = []
                for kc in range(KC):
                    t = xin.tile([P, 512], BF16, tag="xin")
                    nc.sync.dma_start(
                        t[:], xdram[kc * P:(kc + 1) * P, tt * 512:(tt + 1) * 512]
                    )
                    xch.append(t)
                for dt in range(GH):
                    ps = ps512.tile([P, 512], F32, tag="ps512")
                    for kc in range(KC):
                        nc.tensor.matmul(
                            ps[:],
                            lhsT=w_sb[kc][:, dt * P:(dt + 1) * P],
                            rhs=xch[kc][:],
                            start=(kc == 0),
                            stop=(kc == KC - 1),
                        )
                    # evict: out = (psum + b) * scale, bias pre-scaled on host
                    nc.scalar.activation(
                        out_tiles[dt][:, tt * 512:(tt + 1) * 512],
                        ps[:],
                        mybir.ActivationFunctionType.Identity,
                        bias=b_tile[:, dt:dt + 1],
                        scale=scale,
                    )

        # ---- V projection: out [S, dv=512] ----
        for ttg in range(TT):
            xch = []
            for kc in range(KC):
                t = xin.tile([P, 512], BF16, tag="xin")
                nc.sync.dma_start(
                    t[:], xvT[kc * P:(kc + 1) * P, ttg * 512:(ttg + 1) * 512]
                )
                xch.append(t)
            for sub in range(4):
                ps = ps512.tile([P, 512], F32, tag="ps512")
                for kc in range(KC):
                    nc.tensor.matmul(
                        ps[:],
                        lhsT=xch[kc][:, sub * P:(sub + 1) * P],
                        rhs=wv_sb[kc][:],
                        start=(kc == 0),
                        stop=(kc == KC - 1),
                    )
                nc.vector.tensor_add(v_sb[ttg * 4 + sub][:], ps[:], bv_sb[:])

        ctxA.close()

        # ---- attention + output projection, per q tile ----
        ppool = ctx.enter_context(tc.tile_pool(name="p", bufs=2))
        spool = ctx.enter_context(tc.tile_pool(name="sums", bufs=8))
        ps_t = ctx.enter_context(tc.tile_pool(name="ps_t", bufs=2, space="PSUM"))
        ps_o = ctx.enter_context(tc.tile_pool(name="ps_o", bufs=2, space="PSUM"))
        ptp_pool = ctx.enter_context(tc.tile_pool(name="pt", bufs=3))
        at_pool = ctx.enter_context(tc.tile_pool(name="at", bufs=5))
        attn_pool = ctx.enter_context(tc.tile_pool(name="attn", bufs=2))
        ypool = ctx.enter_context(tc.tile_pool(name="ysb", bufs=3))

        for qi in range(QI):
            kv_len = (qi + 1) * P
            nchunks = (kv_len + 511) // 512
            attn_t = attn_pool.tile([P, GD], BF16, tag="attn")
            for h in range(GH):
                p_t = ppool.tile([P, S], BF16, tag="p")
                sums = spool.tile([P, 4], F32, tag="sums")
                for c in range(nchunks):
                    n = min(512, kv_len - c * 512)
                    ps = ps512.tile([P, 512], F32, tag="ps512")
                    nc.tensor.matmul(
                        ps[:, :n],
                        lhsT=qT_sb[h][:, qi * P:(qi + 1) * P],
                        rhs=kT_sb[h][:, c * 512:c * 512 + n],
                        start=True,
                        stop=True,
                    )
                    if c == nchunks - 1:
                        nc.vector.tensor_add(
                            ps[:, n - P:n], ps[:, n - P:n], maskt[:]
                        )
                    nc.scalar.activation(
                        p_t[:, c * 512:c * 512 + n],
                        ps[:, :n],
                        mybir.ActivationFunctionType.Exp,
                        accum_out=sums[:, c:c + 1],
                    )
                tot = spool.tile([P, 1], F32, tag="tot")
                nc.vector.reduce_sum(
                    tot[:], sums[:, :nchunks], axis=mybir.AxisListType.X
                )
                rec = spool.tile([P, 1], F32, tag="rec")
                nc.vector.reciprocal(rec[:], tot[:])

                po = ps_o.tile([P, P], F32)
                for kb in range(qi + 1):
                    ptp = ps_t.tile([P, P], BF16, tag="ptp")
                    nc.tensor.transpose(
                        ptp[:], p_t[:, kb * P:(kb + 1) * P], ident[:]
                    )
                    pts = ptp_pool.tile([P, P], BF16, tag="pt")
                    nc.vector.tensor_copy(pts[:], ptp[:])
                    nc.tensor.matmul(
                        po[:],
                        lhsT=pts[:],
                        rhs=v_sb[kb][:, h * P:(h + 1) * P],
                        start=(kb == 0),
                        stop=(kb == qi),
                    )
                nc.vector.tensor_scalar_mul(
                    attn_t[:, h * P:(h + 1) * P], po[:], rec[:]
                )

            # output projection for this q tile
            ats = []
            for hb in range(GH):
                atp = ps_t.tile([P, P], BF16, tag="ptp")
                nc.tensor.transpose(
                    atp[:], attn_t[:, hb * P:(hb + 1) * P], ident[:]
                )
                a = at_pool.tile([P, P], BF16, tag="at")
                nc.vector.tensor_copy(a[:], atp[:])
                ats.append(a)
            for oc in range(TT):
                ps = ps512.tile([P, 512], F32, tag="ps512")
                for hb in range(GH):
                    nc.tensor.matmul(
                        ps[:],
                        lhsT=ats[hb][:],
                        rhs=wo_sb[hb][:, oc * 512:(oc + 1) * 512],
                        start=(hb == 0),
                        stop=(hb == GH - 1),
                    )
                ysb = ypool.tile([P, 512], F32, tag="y")
                nc.scalar.copy(ysb[:], ps[:])
                nc.sync.dma_start(
                    y[qi * P:(qi + 1) * P, oc * 512:(oc + 1) * 512], ysb[:]
                )
    _split_multi_waits(nc)
    return nc


_NC_CACHE = None


def kernel(**inputs):
    global _NC_CACHE
    from concourse.bass_utils import run_bass_kernel_spmd

    query = np.asarray(inputs["query"], np.float32)
    key = np.asarray(inputs["key"], np.float32)
    value = np.asarray(inputs["value"], np.float32)
    Wq = np.asarray(inputs["Wq"], np.float32)
    bq = np.asarray(inputs["bq"], np.float32)
    Wk = np.asarray(inputs["Wk"], np.float32)
    bk = np.asarray(inputs["bk"], np.float32)
    Wv = np.asarray(inputs["Wv"], np.float32)
    bv = np.asarray(inputs["bv"], np.float32)
    Wo = np.asarray(inputs["Wo"], np.float32)
    bo = np.asarray(inputs["bo"], np.float32)

    # per-batch transposed activations (shared by the 4 cores of a batch)
    xT = {}
    for b in range(B):
        xT[b] = (
            np.ascontiguousarray(query[b].T).astype(NP_BF16),
            np.ascontiguousarray(key[b].T).astype(NP_BF16),
            np.ascontiguousarray(value[b].T).astype(NP_BF16),
        )

    in_maps = []
    for c in range(N_CORES):
        b, g = c // 4, c % 4
        sl = slice(GD * g, GD * (g + 1))
        in_maps.append({
            "xqT": xT[b][0],
            "xkT": xT[b][1],
            "xvT": xT[b][2],
            "wqT": np.ascontiguousarray(Wq[sl, :].T).astype(NP_BF16),
            "wkT": np.ascontiguousarray(Wk[sl, :].T).astype(NP_BF16),
            "wvT": np.ascontiguousarray(Wv[sl, :].T).astype(NP_BF16),
            "woT": np.ascontiguousarray(Wo[:, sl].T).astype(NP_BF16),
            "bq4": np.ascontiguousarray((bq[sl] * SCALE).reshape(GH, P).T),
            "bk4": np.ascontiguousarray(bk[sl].reshape(GH, P).T),
            "bvb": np.ascontiguousarray(
                np.broadcast_to(bv[sl], (P, GD))
            ).astype(np.float32),
        })

    if _NC_CACHE is None:
        _NC_CACHE = build_bass()
    res = run_bass_kernel_spmd(_NC_CACHE, in_maps, list(range(N_CORES)))

    out = np.empty((B, S, D), np.float32)
    for b in range(B):
        acc = res.results[4 * b]["y"].astype(np.float32)
        for g in range(1, 4):
            acc = acc + res.results[4 * b + g]["y"]
        out[b] = acc + bo[None, :]
    return out


# revision 11
# speedup vs baseline: 1.3905x; 1.3905x over previous
"""Trainium2 Bass kernel for KV-cached (causal) multi-head attention.

Full module: y = softmax(mask(QK^T/sqrt(hd))) V  -> out_proj, with
Q/K/V = linear projections of query/key/value inputs.

Shapes (hardcoded): B=2, S=2048, D=2048, H=16 heads, hd=128.

Sharding (8 NeuronCores): core c handles batch b=c//4 and head group
g=c%4 (4 heads = 512 dims).  Each core computes its 4 heads' attention
and a partial output projection y_partial = attn @ Wo[:, g].T; the host
sums the 4 partials per batch and adds bo (the "all-reduce after
out_proj" done host-side at gather time).

On-device layout choices (all matmuls bf16, fp32 PSUM accumulation):
  - host passes transposed activations xT [D, S] and weight slices
    WqT/WkT/WvT = W[g,:].T [D, 512], WoT = Wo[:, g].T [512, D]
  - Q^T, K^T computed as [dq, S] (head dim on partitions) so that
    scores = Q^T.T @ K^T needs no on-device transposes
  - V computed as [S, dv]
  - softmax per q-row (partition) along free kv axis; exp on ScalarE
    with fused per-chunk row-sums (accum_out); causal handled by
    skipping kv blocks beyond the diagonal + one additive mask tile
    on the diagonal 128x128 block
  - P^T for the PV matmul via PE-mode transposes of 128x128 blocks
  - attention output [q, hd] re-transposed per 128-block to feed the
    output projection as lhsT
"""

import sys

for _p in ("/opt/trn_rl_repo",):
    if _p not in sys.path:
        sys.path.insert(0, _p)

from contextlib import ExitStack

import numpy as np
import ml_dtypes

import concourse.bass as bass
import concourse.mybir as mybir
import concourse.tile as tile
from concourse.vector_clock import ScopedClock
from concourse.masks import make_causal_mask, make_identity

BF16 = mybir.dt.bfloat16
F32 = mybir.dt.float32
NP_BF16 = ml_dtypes.bfloat16

B, S, D = 2, 2048, 2048
NH, HD = 16, 128          # total heads, head dim
GH = 4                    # heads per core
GD = GH * HD              # 512 dims per core
P = 128
SCALE = 1.0 / np.sqrt(HD)
N_CORES = 8


def _drain_and_barrier_split(self, tick_clock, wait_clock):
    # The walrus build in this container rejects a Drain carrying more
    # than one sync wait ("Too many sync wait commands").  Semantically
    # equivalent: chain one drain per wait on the sync engine.
    nc = self.nc
    drain_inst = nc.sync.drain()
    wait_clock.add_sem_waits(
        drain_inst.ins, ScopedClock({None: tick_clock.global_clock})
    )
    si = drain_inst.ins.sync_info
    waits = list(si.on_wait)
    if len(waits) > 1:
        drain_inst.ins.sync_info = mybir.SyncInfo(
            on_wait=[waits[0]], on_update=list(si.on_update)
        )
        for w in waits[1:]:
            d = nc.sync.drain()
            d.ins.sync_info = mybir.SyncInfo(on_wait=[w], on_update=[])
    nc.all_engine_barrier()
    assert self.sems is not None
    popped = nc._tile_sem_poison_stack.pop()
    assert popped is self._sem_poison
    nc.clear_and_free_semaphores(list(self.sems.allocated().values()))
    nc.all_engine_barrier()


tile.TileContext._drain_and_barrier = _drain_and_barrier_split


def _split_multi_waits(nc, max_waits=1):
    """This container's walrus rejects instructions carrying more than one
    sync wait.  Hoist extra waits onto same-engine NoOps placed just before
    the instruction (waits execute in engine program order, so this is
    semantically identical)."""
    uid = [0]
    for fn in nc.m.functions:
        for bb in fn.blocks:
            insts = bb.instructions
            new = []
            changed = False
            for inst in insts:
                si = getattr(inst, "sync_info", None)
                waits = list(si.on_wait) if si is not None else []
                if len(waits) > max_waits:
                    changed = True
                    n_keep = max_waits
                    for w in waits[:-n_keep]:
                        nop = mybir.InstNoOp(
                            name=f"WSPLIT-{uid[0]}", ins=[], outs=[]
                        )
                        uid[0] += 1
                        nop.engine = inst.engine
                        nop.sync_info = mybir.SyncInfo(
                            on_wait=[w], on_update=[]
                        )
                        new.append(nop)
                    inst.sync_info = mybir.SyncInfo(
                        on_wait=waits[-n_keep:], on_update=list(si.on_update)
                    )
                new.append(inst)
            if changed:
                bb.instructions = new
    return nc


def build_bass():
    nc = bass.Bass()
    xqT = nc.declare_dram_parameter("xqT", [D, S], BF16, isOutput=False)
    xkT = nc.declare_dram_parameter("xkT", [D, S], BF16, isOutput=False)
    xvT = nc.declare_dram_parameter("xvT", [D, S], BF16, isOutput=False)
    wqT = nc.declare_dram_parameter("wqT", [D, GD], BF16, isOutput=False)
    wkT = nc.declare_dram_parameter("wkT", [D, GD], BF16, isOutput=False)
    wvT = nc.declare_dram_parameter("wvT", [D, GD], BF16, isOutput=False)
    woT = nc.declare_dram_parameter("woT", [GD, D], BF16, isOutput=False)
    bq4 = nc.declare_dram_parameter("bq4", [P, GH], F32, isOutput=False)
    bk4 = nc.declare_dram_parameter("bk4", [P, GH], F32, isOutput=False)
    bvb = nc.declare_dram_parameter("bvb", [P, GD], F32, isOutput=False)
    y = nc.declare_dram_parameter("y", [S, D], F32, isOutput=True)

    KC = D // P               # 16 contraction chunks of 128
    TT = S // 512             # 4 t-tiles of 512
    QI = S // P               # 16 q tiles of 128

    with tile.TileContext(nc) as tc, ExitStack() as ctx:
        const = ctx.enter_context(tc.tile_pool(name="const", bufs=1))
        maskt = const.tile([P, P], F32)
        make_causal_mask(nc, maskt, mask_val=-1e9)
        ident = const.tile([P, P], BF16)
        make_identity(nc, ident)
        bq_sb = const.tile([P, GH], F32)
        nc.sync.dma_start(bq_sb[:], bq4[:])
        bk_sb = const.tile([P, GH], F32)
        nc.sync.dma_start(bk_sb[:], bk4[:])
        bv_sb = const.tile([P, GD], F32)
        nc.sync.dma_start(bv_sb[:], bvb[:])

        # resident weights: 16 chunks of [128, 512] each
        wpool = ctx.enter_context(tc.tile_pool(name="weights", bufs=1))
        wq_sb, wk_sb, wv_sb = [], [], []
        for name, dram, lst in (
            ("wq", wqT, wq_sb), ("wk", wkT, wk_sb), ("wv", wvT, wv_sb)
        ):
            for kc in range(KC):
                t = wpool.tile([P, GD], BF16, name=f"{name}{kc}", tag=f"{name}{kc}")
                nc.sync.dma_start(t[:], dram[kc * P:(kc + 1) * P, :])
                lst.append(t)
        wo_sb = []
        for hb in range(GH):
            t = wpool.tile([P, D], BF16, name=f"woc{hb}", tag=f"wo{hb}")
            nc.sync.dma_start(t[:], woT[hb * P:(hb + 1) * P, :])
            wo_sb.append(t)

        # persistent activations
        act = ctx.enter_context(tc.tile_pool(name="acts", bufs=1))
        qT_sb = [act.tile([P, S], BF16, name=f"qT{h}", tag=f"qT{h}") for h in range(GH)]
        kT_sb = [act.tile([P, S], BF16, name=f"kT{h}", tag=f"kT{h}") for h in range(GH)]
        v_sb = [act.tile([P, GD], BF16, name=f"v{i}", tag=f"v{i}") for i in range(QI)]

        ctxA = ExitStack()
        xin = ctxA.enter_context(tc.tile_pool(name="xin", bufs=24))
        ps512 = ctx.enter_context(
            tc.tile_pool(name="ps512", bufs=4, space="PSUM")
        )

        # ---- Q^T / K^T projections: out [dq=512, S] ----
        for xdram, w_sb, out_tiles, b_tile, scale in (
            (xqT, wq_sb, qT_sb, bq_sb, SCALE),
            (xkT, wk_sb, kT_sb, bk_sb, 1.0),
        ):
            for tt in range(TT):
                xch = []
                for kc in range(KC):
                    t = xin.tile([P, 512], BF16, tag="xin")
                    nc.sync.dma_start(
                        t[:], xdram[kc * P:(kc + 1) * P, tt * 512:(tt + 1) * 512]
                    )
                    xch.append(t)
                for dt in range(GH):
                    ps = ps512.tile([P, 512], F32, tag="ps512")
                    for kc in range(KC):
                        nc.tensor.matmul(
                            ps[:],
                            lhsT=w_sb[kc][:, dt * P:(dt + 1) * P],
                            rhs=xch[kc][:],
                            start=(kc == 0),
                            stop=(kc == KC - 1),
                        )
                    # evict: out = (psum + b) * scale, bias pre-scaled on host
                    nc.scalar.activation(
                        out_tiles[dt][:, tt * 512:(tt + 1) * 512],
                        ps[:],
                        mybir.ActivationFunctionType.Identity,
                        bias=b_tile[:, dt:dt + 1],
                        scale=scale,
                    )

        # ---- V projection: out [S, dv=512] ----
        for ttg in range(TT):
            xch = []
            for kc in range(KC):
                t = xin.tile([P, 512], BF16, tag="xin")
                nc.sync.dma_start(
                    t[:], xvT[kc * P:(kc + 1) * P, ttg * 512:(ttg + 1) * 512]
                )
                xch.append(t)
            for sub in range(4):
                ps = ps512.tile([P, 512], F32, tag="ps512")
                for kc in range(KC):
                    nc.tensor.matmul(
                        ps[:],
                        lhsT=xch[kc][:, sub * P:(sub + 1) * P],
                        rhs=wv_sb[kc][:],
                        start=(kc == 0),
                        stop=(kc == KC - 1),
                    )
                nc.vector.tensor_add(v_sb[ttg * 4 + sub][:], ps[:], bv_sb[:])

        ctxA.close()

        # ---- attention + output projection, per q tile ----
        ppool = ctx.enter_context(tc.tile_pool(name="p", bufs=2))
        spool = ctx.enter_context(tc.tile_pool(name="sums", bufs=8))
        ps_t = ctx.enter_context(tc.tile_pool(name="ps_t", bufs=2, space="PSUM"))
        ps_o = ctx.enter_context(tc.tile_pool(name="ps_o", bufs=2, space="PSUM"))
        ptp_pool = ctx.enter_context(tc.tile_pool(name="pt", bufs=3))
        at_pool = ctx.enter_context(tc.tile_pool(name="at", bufs=5))
        attn_pool = ctx.enter_context(tc.tile_pool(name="attn", bufs=2))
        ypool = ctx.enter_context(tc.tile_pool(name="ysb", bufs=3))

        for qi in range(QI):
            kv_len = (qi + 1) * P
            nchunks = (kv_len + 511) // 512
            attn_t = attn_pool.tile([P, GD], BF16, tag="attn")
            for h in range(GH):
                p_t = ppool.tile([P, S], BF16, tag="p")
                sums = spool.tile([P, 4], F32, tag="sums")
                for c in range(nchunks):
                    n = min(512, kv_len - c * 512)
                    ps = ps512.tile([P, 512], F32, tag="ps512")
                    nc.tensor.matmul(
                        ps[:, :n],
                        lhsT=qT_sb[h][:, qi * P:(qi + 1) * P],
                        rhs=kT_sb[h][:, c * 512:c * 512 + n],
                        start=True,
                        stop=True,
                    )
                    if c == nchunks - 1:
                        nc.vector.tensor_add(
                            ps[:, n - P:n], ps[:, n - P:n], maskt[:]
                        )
                    nc.scalar.activation(
                        p_t[:, c * 512:c * 512 + n],
                        ps[:, :n],
                        mybir.ActivationFunctionType.Exp,
                        accum_out=sums[:, c:c + 1],
                    )
                tot = spool.tile([P, 1], F32, tag="tot")
                nc.vector.reduce_sum(
                    tot[:], sums[:, :nchunks], axis=mybir.AxisListType.X
                )
                rec = spool.tile([P, 1], F32, tag="rec")
                nc.vector.reciprocal(rec[:], tot[:])

                po = ps_o.tile([P, P], F32)
                for kb in range(qi + 1):
                    ptp = ps_t.tile([P, P], BF16, tag="ptp")
                    nc.tensor.transpose(
                        ptp[:], p_t[:, kb * P:(kb + 1) * P], ident[:]
                    )
                    pts = ptp_pool.tile([P, P], BF16, tag="pt")
                    nc.vector.tensor_copy(pts[:], ptp[:])
                    nc.tensor.matmul(
                        po[:],
                        lhsT=pts[:],
                        rhs=v_sb[kb][:, h * P:(h + 1) * P],
                        start=(kb == 0),
                        stop=(kb == qi),
                    )
                nc.vector.tensor_scalar_mul(
                    attn_t[:, h * P:(h + 1) * P], po[:], rec[:]
                )

            # output projection for this q tile
            ats = []
            for hb in range(GH):
                atp = ps_t.tile([P, P], BF16, tag="ptp")
                nc.tensor.transpose(
                    atp[:], attn_t[:, hb * P:(hb + 1) * P], ident[:]
                )
                a = at_pool.tile([P, P], BF16, tag="at")
                nc.vector.tensor_copy(a[:], atp[:])
                ats.append(a)
            for oc in range(TT):
                ps = ps512.tile([P, 512], F32, tag="ps512")
                for hb in range(GH):
                    nc.tensor.matmul(
                        ps[:],
                        lhsT=ats[hb][:],
                        rhs=wo_sb[hb][:, oc * 512:(oc + 1) * 512],
                        start=(hb == 0),
                        stop=(hb == GH - 1),
                    )
                ysb = ypool.tile([P, 512], F32, tag="y")
                nc.scalar.copy(ysb[:], ps[:])
                nc.sync.dma_start(
                    y[qi * P:(qi + 1) * P, oc * 512:(oc + 1) * 512], ysb[:]
                )
    _split_multi_waits(nc)
    return nc


_NC_CACHE = None
_last_in_maps = None


def kernel(**inputs):
    global _NC_CACHE
    from concourse.bass_utils import run_bass_kernel_spmd

    query = np.asarray(inputs["query"], np.float32)
    key = np.asarray(inputs["key"], np.float32)
    value = np.asarray(inputs["value"], np.float32)
    Wq = np.asarray(inputs["Wq"], np.float32)
    bq = np.asarray(inputs["bq"], np.float32)
    Wk = np.asarray(inputs["Wk"], np.float32)
    bk = np.asarray(inputs["bk"], np.float32)
    Wv = np.asarray(inputs["Wv"], np.float32)
    bv = np.asarray(inputs["bv"], np.float32)
    Wo = np.asarray(inputs["Wo"], np.float32)
    bo = np.asarray(inputs["bo"], np.float32)

    # per-batch transposed activations (shared by the 4 cores of a batch)
    xT = {}
    for b in range(B):
        xT[b] = (
            query[b].T.astype(NP_BF16),
            key[b].T.astype(NP_BF16),
            value[b].T.astype(NP_BF16),
        )

    in_maps = []
    for c in range(N_CORES):
        b, g = c // 4, c % 4
        sl = slice(GD * g, GD * (g + 1))
        in_maps.append({
            "xqT": xT[b][0],
            "xkT": xT[b][1],
            "xvT": xT[b][2],
            "wqT": Wq[sl, :].T.astype(NP_BF16),
            "wkT": Wk[sl, :].T.astype(NP_BF16),
            "wvT": Wv[sl, :].T.astype(NP_BF16),
            "woT": Wo[:, sl].T.astype(NP_BF16),
            "bq4": np.ascontiguousarray((bq[sl] * SCALE).reshape(GH, P).T),
            "bk4": np.ascontiguousarray(bk[sl].reshape(GH, P).T),
            "bvb": np.ascontiguousarray(
                np.broadcast_to(bv[sl], (P, GD))
            ).astype(np.float32),
        })

    global _last_in_maps
    _last_in_maps = in_maps
    if _NC_CACHE is None:
        _NC_CACHE = build_bass()
    res = run_bass_kernel_spmd(_NC_CACHE, in_maps, list(range(N_CORES)))

    out = np.empty((B, S, D), np.float32)
    for b in range(B):
        acc = res.results[4 * b]["y"].astype(np.float32)
        for g in range(1, 4):
            acc = acc + res.results[4 * b + g]["y"]
        out[b] = acc + bo[None, :]
    return out
